# revision 70
# baseline (speedup 1.0000x reference)
"""Trainium2 Bass kernel for the CAA (channel-affinity attention) module.

Reference computation per sample b (C=1024 channels, N=256 positions):
    x_hat = x^T                              (N, C)
    q = relu(BN1(Wq @ x_hat))                (64, C)
    k = relu(BN2(Wk @ x_hat))                (64, C)
    sim[c, d] = sum_o k[o, c] * q[o, d]      (C, C)
    aff = softmax(rowmax(sim) - sim, axis d) == softmax(-sim, axis d)
    v = relu(BN3(Wv @ x))                    (C, N)
    out = alpha * (aff @ v) + x              (C, N)

Device-side strategy (pure data parallel, 4 samples per core x 8 cores):
  * BN folded into weights/bias on the host; |alpha| folded into Wv/t3.
  * qk and v projections and the aff@v contraction run in fp8 with the
    DoubleRow perf mode (2 contraction rows per PE column-cycle), halving
    their PE time vs bf16.  sim (contraction 64) stays bf16.
  * the exp(-sim) operand of aff@v is stored as fp8 e5m2 (22-nat dynamic
    range).  Per-column (c) softmax range alignment comes free by centering
    q (qt = q - mean_d q): any per-c scale of E cancels in U/Z.  v is
    quantized to fp8 e4m3 with a single generous global scale VMAX (fp8 is
    floating point, so headroom costs no precision); the matching constant
    enters the exp bias and the "ones" Z-columns and cancels in U/Z.
  * per-sample phases are interleaved chunk-wise (v-chunk j, sim-chunk j-2,
    u-chunk j of the previous sample) to keep PE/ACT/DVE all busy.
"""

import math
import os
import sys

import numpy as np

_REPO = "/opt/trn_rl_repo"
if _REPO not in sys.path:
    sys.path.insert(0, _REPO)

import ml_dtypes  # noqa: E402

import concourse.bacc as bacc  # noqa: E402
import concourse.tile as tile  # noqa: E402
from concourse import mybir  # noqa: E402
from concourse.bass_utils import run_bass_kernel_spmd  # noqa: E402

F32 = mybir.dt.float32
BF16 = mybir.dt.bfloat16
FP8 = mybir.dt.float8e4
FP8E5 = mybir.dt.float8e5
AF = mybir.ActivationFunctionType
ALU = mybir.AluOpType
DR = mybir.MatmulPerfMode.DoubleRow
BFNP = ml_dtypes.bfloat16
F8NP = ml_dtypes.float8_e4m3

B, C, N = 32, 1024, 256
DQ = 64
NCORES = 8
BS = B // NCORES  # samples per core
CCH = C // 128    # chunks of the channel dim
KCH = N // 128    # chunks of the position dim (qk contraction)
EPS = 1e-5

GAMMA = 13.0      # global exp shift (keeps E in e5m2 range; numpy-validated)
GCONST = 48.0     # global scale on b_d

LAST_RESULTS = None  # BassKernelResults of the most recent run
_NC_CACHE = {}


def _build(bs: int = BS):
    nc = bacc.Bacc("TRN2", target_bir_lowering=False, debug=False)

    x_d = nc.dram_tensor("x_in", (bs, 128, CCH, N), F32, kind="ExternalInput")
    xq_d = nc.dram_tensor("xq_in", (bs, 128, CCH, N), FP8, kind="ExternalInput")
    xt_d = nc.dram_tensor("xt_in", (bs, 128, KCH, C), FP8, kind="ExternalInput")
    wqkt_d = nc.dram_tensor("wqkt", (128, KCH, 128), FP8, kind="ExternalInput")
    wvt_d = nc.dram_tensor("wvt", (128, CCH, C), FP8, kind="ExternalInput")
    # packed fp32 consts: qsc | tqk | vsc[8] | t3[8] | ebias
    cst_d = nc.dram_tensor("cst", (128, 19), F32, kind="ExternalInput")
    vcol_d = nc.dram_tensor("vcol", (128, CCH, 2), FP8, kind="ExternalInput")
    out_d = nc.dram_tensor("y_out", (bs, 128, CCH, N), F32, kind="ExternalOutput")

    with tile.TileContext(nc) as tc:
        with (
            tc.tile_pool(name="consts", bufs=1) as consts,
            # 3 bufs: the delayed u-epilogue still reads x of sample b-1
            # after load_x(b+1) has been emitted
            tc.tile_pool(name="xp", bufs=3) as xp,
            tc.tile_pool(name="xqp", bufs=3) as xqp,
            tc.tile_pool(name="xtp", bufs=3) as xtp,
            tc.tile_pool(name="qkp", bufs=2) as qkp,
            tc.tile_pool(name="qtp", bufs=2) as qtp,
            tc.tile_pool(name="vtp", bufs=3) as vtp,
            tc.tile_pool(name="k0p", bufs=2) as k0p,
            tc.tile_pool(name="etp", bufs=8) as etp,
            tc.tile_pool(name="vp", bufs=2) as vp,
            tc.tile_pool(name="outp", bufs=4) as outp,
            tc.tile_pool(name="smallp", bufs=12) as smallp,
            tc.tile_pool(name="psbig", bufs=2, space="PSUM") as psbig,
            tc.tile_pool(name="psv", bufs=2, space="PSUM") as psv,
            tc.tile_pool(name="psu", bufs=2, space="PSUM") as psu,
        ):
            # weights via SWDGE (gpsimd) so they don't block the sync queue;
            # all small fp32 consts ride in one packed [128, 19] transfer.
            # cst rides the sync ring first: it gates the first relu chain.
            cst = consts.tile([128, 19], F32, tag="cst")
            nc.sync.dma_start(out=cst, in_=cst_d[:])
            wqkt = consts.tile([128, KCH, 128], FP8, tag="wqkt")
            nc.gpsimd.dma_start(out=wqkt, in_=wqkt_d[:])
            qsc = cst[:, 0:1]
            tqk = cst[:, 1:2]
            vsc = cst[:, 2:2 + CCH]
            t3 = cst[:, 10:10 + CCH]
            ebias = cst[:, 18:19]
            wvt = consts.tile([128, CCH, C], FP8, tag="wvt")
            for h in range(0, CCH, 2):
                nc.gpsimd.dma_start(out=wvt[:, h:h + 2, :], in_=wvt_d[:, h:h + 2, :])
            vcol = consts.tile([128, CCH, 2], FP8, tag="vcol")
            nc.gpsimd.dma_start(out=vcol, in_=vcol_d[:])
            zero = consts.tile([128, 1], F32, tag="zero")
            nc.vector.memset(zero, 0.0)
            # touch the activation table early so the lazy ACT_TABLE_LOAD
            # doesn't delay the first (critical-path) op
            warm = consts.tile([128, 1], F32, tag="warm")
            nc.scalar.activation(out=warm, in_=zero, func=AF.Exp,
                                 bias=zero[:, 0:1], scale=1.0)

            x_sb = [None] * bs
            xq_sb = [None] * bs
            xt_sb = [None] * bs
            qt_sb = [None] * bs
            k0 = [None] * bs
            v_sb = [None] * bs
            et = [None] * bs

            def load_xt(b):
                # two half-loads: qk's first matmul only waits on the first
                xt_sb[b] = xtp.tile([128, KCH, C], FP8, tag="xt", name=f"xt{b}")
                for h in range(2):
                    nc.sync.dma_start(out=xt_sb[b][:, :, h * 512:(h + 1) * 512],
                                      in_=xt_d[b, :, :, h * 512:(h + 1) * 512])

            def load_xq(b):
                xq_sb[b] = xqp.tile([128, CCH, N], FP8, tag="xq", name=f"xq{b}")
                nc.sync.dma_start(out=xq_sb[b], in_=xq_d[b])

            def load_x(b):
                # residual load is latency-tolerant: keep it off the sync
                # ring so it can't delay the next sample's fp8 loads
                x_sb[b] = xp.tile([128, CCH, N], F32, tag="x", name=f"x{b}")
                nc.gpsimd.dma_start(out=x_sb[b], in_=x_d[b])

            def qk_phase(b):
                # q/k projection in fp8 DoubleRow: psum rows 0:64 = q, 64:128 = k.
                # Two [128, 512] half-psums from the v pool keep qk out of the
                # sim psum rotation (no slot contention at sample boundaries).
                qk_sb = qkp.tile([128, C], BF16, tag="qk", name=f"qk{b}")
                qs = [None, None]
                for cb in range(C // 512):
                    qk_ps = psv.tile([128, 512], F32, tag="psv")
                    nc.tensor.matmul(
                        qk_ps,
                        wqkt[:, 0:KCH, :],
                        xt_sb[b][:, 0:KCH, cb * 512:(cb + 1) * 512],
                        start=True, stop=True, perf_mode=DR,
                    )
                    qs[cb] = smallp.tile([128, 1], F32, tag="qs",
                                         name=f"qs{b}_{cb}")
                    nc.scalar.activation(
                        out=qk_sb[:, cb * 512:(cb + 1) * 512], in_=qk_ps,
                        func=AF.Relu,
                        bias=tqk[:, 0:1], scale=qsc[:, 0:1], accum_out=qs[cb],
                    )
                # qt = q - mean_d(q): centers sim rows so exp fits e5m2
                qbar = smallp.tile([128, 1], F32, tag="qbar")
                nc.vector.tensor_scalar(
                    out=qbar[0:DQ], in0=qs[0][0:DQ], scalar1=qs[1][0:DQ],
                    scalar2=1.0 / C, op0=ALU.add, op1=ALU.mult,
                )
                qt_sb[b] = qtp.tile([DQ, C], BF16, tag="qt", name=f"qt{b}")
                nc.vector.tensor_scalar(
                    out=qt_sb[b], in0=qk_sb[0:DQ, :], scalar1=qbar[0:DQ],
                    scalar2=None, op0=ALU.subtract,
                )
                # k must sit at partition base 0 to act as matmul rhs
                k0[b] = k0p.tile([DQ, C], BF16, tag="k0", name=f"k0_{b}")
                nc.sync.dma_start(out=k0[b], in_=qk_sb[DQ:128, :])
                # v tile for this sample; Z-accumulator columns are constant
                v_sb[b] = vp.tile([128, CCH, N + 2], FP8, tag="v", name=f"v{b}")
                nc.gpsimd.dma_start(out=v_sb[b][:, :, N:N + 2], in_=vcol[:])
                et[b] = [None] * (CCH // 2)

            def v_chunk(b, m):
                # v_q = relu(Wv' @ x + t3) * (240/VMAX), fp8; scales folded.
                # Alternate the epilogue between ACT (1 op) and DVE (2 ops)
                # so neither engine becomes the per-iteration bottleneck.
                v_ps = psv.tile([128, 512], F32, tag="psv")
                for kp in range(CCH // 2):
                    nc.tensor.matmul(
                        v_ps[:, 0:N],
                        wvt[:, 2 * kp:2 * kp + 2, m * 128:(m + 1) * 128],
                        xq_sb[b][:, 2 * kp:2 * kp + 2, :],
                        start=(kp == 0), stop=(kp == CCH // 2 - 1),
                        perf_mode=DR,
                    )
                if m % 4 == 3:
                    nc.scalar.activation(
                        out=v_sb[b][:, m, 0:N], in_=v_ps[:, 0:N], func=AF.Relu,
                        bias=t3[:, m:m + 1], scale=vsc[:, m:m + 1],
                    )
                else:
                    vt = vtp.tile([128, N], BF16, tag="vt")
                    nc.vector.tensor_scalar(
                        out=vt, in0=v_ps[:, 0:N], scalar1=vsc[:, m:m + 1],
                        scalar2=t3[:, m:m + 1], op0=ALU.mult, op1=ALU.add,
                    )
                    nc.vector.tensor_scalar(
                        out=v_sb[b][:, m, 0:N], in0=vt, scalar1=0.0,
                        scalar2=None, op0=ALU.max,
                    )

            def sim_chunk(b, d):
                # et[d//2][:, d%2, c] = exp(ebias - sim[c, d]) in e5m2
                s_ps = psbig.tile([128, C], F32, tag="psbig")
                for cb in range(C // 512):
                    nc.tensor.matmul(
                        s_ps[:, cb * 512:(cb + 1) * 512],
                        qt_sb[b][:, d * 128:(d + 1) * 128],
                        k0[b][:, cb * 512:(cb + 1) * 512],
                        start=True, stop=True,
                    )
                if d % 2 == 0:
                    et[b][d // 2] = etp.tile([128, 2, C], FP8E5, tag="et",
                                             name=f"et{b}_{d // 2}")
                nc.scalar.activation(
                    out=et[b][d // 2][:, d % 2, :], in_=s_ps, func=AF.Exp,
                    bias=ebias[:, 0:1], scale=-1.0,
                )

            # u epilogues run one iteration behind their matmuls so the DVE
            # never reaches an STT whose psum is still accumulating (in-order
            # queue head-of-line blocking would stall the v-affine chain)
            u_pending = []
            u_osb = [None]

            def u_mm(b, m):
                # U = E @ v_ext
                u_ps = psu.tile([128, N + 2], F32, tag="psu",
                                name=f"ups{b}_{m}")
                for kp in range(CCH // 2):
                    nc.tensor.matmul(
                        u_ps,
                        et[b][kp][:, :, m * 128:(m + 1) * 128],
                        v_sb[b][:, 2 * kp:2 * kp + 2, :],
                        start=(kp == 0), stop=(kp == CCH // 2 - 1),
                        perf_mode=DR,
                    )
                u_pending.append((b, m, u_ps))

            def u_epi(keep):
                # out = U * (1/Z) + x fused on DVE, then stream the result out
                while len(u_pending) > keep:
                    b, m, u_ps = u_pending.pop(0)
                    if m % 2 == 0:
                        u_osb[0] = outp.tile([128, 2, N], F32, tag="o",
                                             name=f"o{b}_{m // 2}")
                    o_sb = u_osb[0]
                    rz = smallp.tile([128, 1], F32, tag="rz")
                    nc.vector.reciprocal(out=rz, in_=u_ps[:, N:N + 1])
                    nc.vector.scalar_tensor_tensor(
                        out=o_sb[:, m % 2, :],
                        in0=u_ps[:, 0:N],
                        scalar=rz[:, 0:1],
                        in1=x_sb[b][:, m, :],
                        op0=ALU.mult,
                        op1=ALU.add,
                    )
                    if m % 2 == 1:
                        nc.sync.dma_start(
                            out=out_d[b, :, m - 1:m + 1, :],
                            in_=o_sb,
                        )

            def sample(b, with_u):
                # qk(b) was already emitted by the previous sample's tail
                # (so its relu isn't queued behind the trailing exps)
                last = b == bs - 1
                for j in range(CCH):
                    # sim first so its exp leads the ACT queue each iteration
                    if j >= 1:
                        sim_chunk(b, j - 1)
                    if last and j == CCH - 1:
                        # finish the trailing sim early so the final
                        # u-phase does not wait on exp at the very end
                        sim_chunk(b, CCH - 1)
                    # v chunks 0/1 were pre-emitted at the previous sample's
                    # boundary (PE filler while the trailing exps drain)
                    if j < CCH - 2:
                        v_chunk(b, j + 2)
                    if j == 0 and b + 1 < bs:
                        load_xt(b + 1)
                    if j == 1 and b + 1 < bs:
                        load_xq(b + 1)
                    if j == 3 and b + 1 < bs:
                        load_x(b + 1)
                    if with_u:
                        u_mm(b - 1, j)
                        u_epi(keep=0)
                if not last:
                    qk_phase(b + 1)
                    v_chunk(b + 1, 0)
                    v_chunk(b + 1, 1)
                    sim_chunk(b, CCH - 1)

            load_xt(0)
            load_xq(0)
            load_x(0)
            qk_phase(0)
            v_chunk(0, 0)
            v_chunk(0, 1)
            for b in range(bs):
                sample(b, with_u=(b > 0))
            for j in range(CCH):
                u_mm(bs - 1, j)
                u_epi(keep=0)

    nc.compile()
    return nc


def _prep_host(x, Wq, Wk, Wv, bn1_g, bn1_b, bn1_m, bn1_v,
               bn2_g, bn2_b, bn2_m, bn2_v, bn3_g, bn3_b, bn3_m, bn3_v,
               abs_alpha):
    f = np.float32
    s1 = (bn1_g / np.sqrt(bn1_v + EPS)).astype(f)
    t1 = (bn1_b - s1 * bn1_m).astype(f)
    s2 = (bn2_g / np.sqrt(bn2_v + EPS)).astype(f)
    t2 = (bn2_b - s2 * bn2_m).astype(f)
    s3u = (bn3_g / np.sqrt(bn3_v + EPS)).astype(f)
    s3 = s3u * np.float32(abs_alpha)
    t3 = ((bn3_b - s3u * bn3_m) * abs_alpha).astype(f)

    x = np.asarray(x, dtype=f)
    sx = np.float32(240.0) / np.float32(np.abs(x).max() + 1e-30)

    # qk weights: per-row fp8 quant, dequant scale folded into the relu
    wqk = np.concatenate([Wq * s1[:, None], Wk * s2[:, None]], axis=0).astype(f)
    uqk = np.maximum(np.abs(wqk).max(axis=1) / 240.0, 1e-30).astype(f)
    wqk_q = np.clip(wqk / uqk[:, None], -240.0, 240.0).astype(F8NP)
    wqkt = np.ascontiguousarray(
        wqk_q.T.reshape(KCH, 128, 128).transpose(1, 0, 2))
    qsc = (uqk / sx).reshape(128, 1).astype(f)
    tqk = np.concatenate([t1, t2]).reshape(128, 1).astype(f)

    # v weights: per-row fp8 quant; global output scale 240/VMAX folded in
    vmax = np.float32(max(1.0, 40.0 * abs_alpha))
    vqs = np.float32(240.0) / vmax
    wv2 = (Wv * s3[:, None]).astype(f)
    uv = np.maximum(np.abs(wv2).max(axis=1) / 240.0, 1e-30).astype(f)
    wv_q = np.clip(wv2 / uv[:, None], -240.0, 240.0).astype(F8NP)
    wvt = np.ascontiguousarray(
        wv_q.T.reshape(CCH, 128, C).transpose(1, 0, 2))
    vsc = np.ascontiguousarray(
        ((uv / sx) * vqs).reshape(CCH, 128).T).astype(f)
    t3r = np.ascontiguousarray((t3 * vqs).reshape(CCH, 128).T).astype(f)
    ebias = np.full((128, 1),
                    -GAMMA + math.log(GCONST * vmax / 240.0), dtype=f)
    cst = np.concatenate([qsc, tqk, vsc, t3r, ebias], axis=1).astype(f)

    x8 = np.clip(x * sx, -240.0, 240.0).astype(F8NP)
    # residual [b, p(=c local), kc, n]
    xr = np.ascontiguousarray(x.reshape(B, CCH, 128, N).transpose(0, 2, 1, 3))
    xq8 = np.ascontiguousarray(x8.reshape(B, CCH, 128, N).transpose(0, 2, 1, 3))
    # [b, p(=n local), kc, c]
    xt8 = np.ascontiguousarray(
        x8.transpose(0, 2, 1).reshape(B, KCH, 128, C).transpose(0, 2, 1, 3))
    return xr, xq8, xt8, wqkt, wvt, cst, vqs


def kernel(x, Wq, Wk, Wv,
           bn1_g, bn1_b, bn1_m, bn1_v,
           bn2_g, bn2_b, bn2_m, bn2_v,
           bn3_g, bn3_b, bn3_m, bn3_v,
           alpha):
    global LAST_RESULTS
    args = [np.asarray(a, dtype=np.float32) for a in (
        x, Wq, Wk, Wv, bn1_g, bn1_b, bn1_m, bn1_v,
        bn2_g, bn2_b, bn2_m, bn2_v, bn3_g, bn3_b, bn3_m, bn3_v)]
    alpha_val = float(np.asarray(alpha).reshape(-1)[0])
    if alpha_val == 0.0:
        return np.asarray(x, dtype=np.float32).copy()

    xr, xq8, xt8, wqkt, wvt, cst, vqs = _prep_host(*args, abs(alpha_val))
    # Z-accumulator columns carry sign(alpha) * (240/VMAX) = sgn/s_v
    vcol = np.full((128, CCH, 2), np.sign(alpha_val) * vqs, dtype=F8NP)

    if "nc" not in _NC_CACHE:
        _NC_CACHE["nc"] = _build()
    nc = _NC_CACHE["nc"]

    in_maps = []
    for cid in range(NCORES):
        sl = slice(cid * BS, (cid + 1) * BS)
        in_maps.append({
            "x_in": np.ascontiguousarray(xr[sl]),
            "xq_in": np.ascontiguousarray(xq8[sl]),
            "xt_in": np.ascontiguousarray(xt8[sl]),
            "wqkt": wqkt,
            "wvt": wvt,
            "cst": cst,
            "vcol": vcol,
        })

    trace = bool(int(os.environ.get("KERNEL_TRACE", "0")))
    tmpdir = os.environ.get("KERNEL_TRACE_DIR") or None
    for attempt in range(3):
        res = run_bass_kernel_spmd(
            nc, in_maps, core_ids=list(range(NCORES)),
            trace=trace and attempt == 0, tmpdir=tmpdir,
        )
        LAST_RESULTS = res
        y = np.concatenate(
            [res.results[cid]["y_out"] for cid in range(NCORES)], axis=0)
        y = y.transpose(0, 2, 1, 3).reshape(B, C, N)
        if np.isfinite(y).all():
            break
        # rare transient NaN flake on device: rerun
    return np.ascontiguousarray(y.astype(np.float32))


# revision 71
# speedup vs baseline: 1.0239x; 1.0239x over previous
"""Trainium2 Bass kernel for the CAA (channel-affinity attention) module.

Reference computation per sample b (C=1024 channels, N=256 positions):
    x_hat = x^T                              (N, C)
    q = relu(BN1(Wq @ x_hat))                (64, C)
    k = relu(BN2(Wk @ x_hat))                (64, C)
    sim[c, d] = sum_o k[o, c] * q[o, d]      (C, C)
    aff = softmax(rowmax(sim) - sim, axis d) == softmax(-sim, axis d)
    v = relu(BN3(Wv @ x))                    (C, N)
    out = alpha * (aff @ v) + x              (C, N)

Device-side strategy (pure data parallel, 4 samples per core x 8 cores):
  * BN folded into weights/bias on the host; |alpha| folded into Wv/t3.
  * qk and v projections and the aff@v contraction run in fp8 with the
    DoubleRow perf mode (2 contraction rows per PE column-cycle), halving
    their PE time vs bf16.  sim (contraction 64) stays bf16.
  * the exp(-sim) operand of aff@v is stored as fp8 e5m2 (22-nat dynamic
    range).  Per-column (c) softmax range alignment comes free by centering
    q (qt = q - mean_d q): any per-c scale of E cancels in U/Z.  v is
    quantized to fp8 e4m3 with a single generous global scale VMAX (fp8 is
    floating point, so headroom costs no precision); the matching constant
    enters the exp bias and the "ones" Z-columns and cancels in U/Z.
  * per-sample phases are interleaved chunk-wise (v-chunk j, sim-chunk j-2,
    u-chunk j of the previous sample) to keep PE/ACT/DVE all busy.
"""

import math
import os
import sys

import numpy as np

_REPO = "/opt/trn_rl_repo"
if _REPO not in sys.path:
    sys.path.insert(0, _REPO)

import ml_dtypes  # noqa: E402

import concourse.bacc as bacc  # noqa: E402
import concourse.tile as tile  # noqa: E402
from concourse import mybir  # noqa: E402
from concourse.bass_utils import run_bass_kernel_spmd  # noqa: E402

F32 = mybir.dt.float32
BF16 = mybir.dt.bfloat16
FP8 = mybir.dt.float8e4
FP8E5 = mybir.dt.float8e5
AF = mybir.ActivationFunctionType
ALU = mybir.AluOpType
DR = mybir.MatmulPerfMode.DoubleRow
BFNP = ml_dtypes.bfloat16
F8NP = ml_dtypes.float8_e4m3

B, C, N = 32, 1024, 256
DQ = 64
NCORES = 8
BS = B // NCORES  # samples per core
CCH = C // 128    # chunks of the channel dim
KCH = N // 128    # chunks of the position dim (qk contraction)
EPS = 1e-5

GAMMA = 13.0      # global exp shift (keeps E in e5m2 range; numpy-validated)
GCONST = 48.0     # global scale on b_d

LAST_RESULTS = None  # BassKernelResults of the most recent run
_NC_CACHE = {}


def _build(bs: int = BS):
    nc = bacc.Bacc("TRN2", target_bir_lowering=False, debug=False)

    x_d = nc.dram_tensor("x_in", (bs, 128, CCH, N), F32, kind="ExternalInput")
    xq_d = nc.dram_tensor("xq_in", (bs, 128, CCH, N), FP8, kind="ExternalInput")
    xt_d = nc.dram_tensor("xt_in", (bs, 128, KCH, C), FP8, kind="ExternalInput")
    wqkt_d = nc.dram_tensor("wqkt", (128, KCH, 128), FP8, kind="ExternalInput")
    wvt_d = nc.dram_tensor("wvt", (128, CCH, C), FP8, kind="ExternalInput")
    # packed fp32 consts: qsc | tqk | vsc[8] | t3[8] | ebias
    cst_d = nc.dram_tensor("cst", (128, 19), F32, kind="ExternalInput")
    vcol_d = nc.dram_tensor("vcol", (128, CCH, 2), FP8, kind="ExternalInput")
    out_d = nc.dram_tensor("y_out", (bs, 128, CCH, N), F32, kind="ExternalOutput")

    with tile.TileContext(nc) as tc:
        with (
            tc.tile_pool(name="consts", bufs=1) as consts,
            # 3 bufs: the delayed u-epilogue still reads x of sample b-1
            # after load_x(b+1) has been emitted
            tc.tile_pool(name="xp", bufs=3) as xp,
            tc.tile_pool(name="xqp", bufs=3) as xqp,
            tc.tile_pool(name="xtp", bufs=3) as xtp,
            tc.tile_pool(name="qkp", bufs=2) as qkp,
            tc.tile_pool(name="qtp", bufs=2) as qtp,
            tc.tile_pool(name="vtp", bufs=3) as vtp,
            tc.tile_pool(name="k0p", bufs=2) as k0p,
            tc.tile_pool(name="etp", bufs=8) as etp,
            tc.tile_pool(name="vp", bufs=2) as vp,
            tc.tile_pool(name="outp", bufs=4) as outp,
            tc.tile_pool(name="smallp", bufs=12) as smallp,
            tc.tile_pool(name="psbig", bufs=2, space="PSUM") as psbig,
            tc.tile_pool(name="psv", bufs=2, space="PSUM") as psv,
            tc.tile_pool(name="psu", bufs=2, space="PSUM") as psu,
        ):
            # weights via SWDGE (gpsimd) so they don't block the sync queue;
            # all small fp32 consts ride in one packed [128, 19] transfer.
            # cst rides the sync ring first: it gates the first relu chain.
            cst = consts.tile([128, 19], F32, tag="cst")
            nc.sync.dma_start(out=cst, in_=cst_d[:])
            wqkt = consts.tile([128, KCH, 128], FP8, tag="wqkt")
            nc.gpsimd.dma_start(out=wqkt, in_=wqkt_d[:])
            qsc = cst[:, 0:1]
            tqk = cst[:, 1:2]
            vsc = cst[:, 2:2 + CCH]
            t3 = cst[:, 10:10 + CCH]
            ebias = cst[:, 18:19]
            wvt = consts.tile([128, CCH, C], FP8, tag="wvt")
            for h in range(0, CCH, 2):
                nc.gpsimd.dma_start(out=wvt[:, h:h + 2, :], in_=wvt_d[:, h:h + 2, :])
            vcol = consts.tile([128, CCH, 2], FP8, tag="vcol")
            nc.gpsimd.dma_start(out=vcol, in_=vcol_d[:])
            zero = consts.tile([128, 1], F32, tag="zero")
            nc.vector.memset(zero, 0.0)
            # touch the activation table early so the lazy ACT_TABLE_LOAD
            # doesn't delay the first (critical-path) op
            warm = consts.tile([128, 1], F32, tag="warm")
            nc.scalar.activation(out=warm, in_=zero, func=AF.Exp,
                                 bias=zero[:, 0:1], scale=1.0)

            x_sb = [None] * bs
            xq_sb = [None] * bs
            xt_sb = [None] * bs
            qt_sb = [None] * bs
            k0 = [None] * bs
            v_sb = [None] * bs
            et = [None] * bs

            def load_xt(b):
                # two half-loads: qk's first matmul only waits on the first
                xt_sb[b] = xtp.tile([128, KCH, C], FP8, tag="xt", name=f"xt{b}")
                for h in range(2):
                    nc.sync.dma_start(out=xt_sb[b][:, :, h * 512:(h + 1) * 512],
                                      in_=xt_d[b, :, :, h * 512:(h + 1) * 512])

            def load_xq(b):
                xq_sb[b] = xqp.tile([128, CCH, N], FP8, tag="xq", name=f"xq{b}")
                nc.sync.dma_start(out=xq_sb[b], in_=xq_d[b])

            def load_x(b):
                # residual load is latency-tolerant: keep it off the sync
                # ring so it can't delay the next sample's fp8 loads
                x_sb[b] = xp.tile([128, CCH, N], F32, tag="x", name=f"x{b}")
                nc.gpsimd.dma_start(out=x_sb[b], in_=x_d[b])

            def qk_phase(b):
                # q/k projection in fp8 DoubleRow: psum rows 0:64 = q, 64:128 = k.
                # Two [128, 512] half-psums from the v pool keep qk out of the
                # sim psum rotation (no slot contention at sample boundaries).
                qk_sb = qkp.tile([128, C], BF16, tag="qk", name=f"qk{b}")
                qs = [None, None]
                for cb in range(C // 512):
                    qk_ps = psv.tile([128, 512], F32, tag="psv")
                    nc.tensor.matmul(
                        qk_ps,
                        wqkt[:, 0:KCH, :],
                        xt_sb[b][:, 0:KCH, cb * 512:(cb + 1) * 512],
                        start=True, stop=True, perf_mode=DR,
                    )
                    qs[cb] = smallp.tile([128, 1], F32, tag="qs",
                                         name=f"qs{b}_{cb}")
                    nc.scalar.activation(
                        out=qk_sb[:, cb * 512:(cb + 1) * 512], in_=qk_ps,
                        func=AF.Relu,
                        bias=tqk[:, 0:1], scale=qsc[:, 0:1], accum_out=qs[cb],
                    )
                # qt = q - mean_d(q): centers sim rows so exp fits e5m2
                qbar = smallp.tile([128, 1], F32, tag="qbar")
                nc.vector.tensor_scalar(
                    out=qbar[0:DQ], in0=qs[0][0:DQ], scalar1=qs[1][0:DQ],
                    scalar2=1.0 / C, op0=ALU.add, op1=ALU.mult,
                )
                qt_sb[b] = qtp.tile([DQ, C], BF16, tag="qt", name=f"qt{b}")
                nc.vector.tensor_scalar(
                    out=qt_sb[b], in0=qk_sb[0:DQ, :], scalar1=qbar[0:DQ],
                    scalar2=None, op0=ALU.subtract,
                )
                # k must sit at partition base 0 to act as matmul rhs
                k0[b] = k0p.tile([DQ, C], BF16, tag="k0", name=f"k0_{b}")
                nc.sync.dma_start(out=k0[b], in_=qk_sb[DQ:128, :])
                # v tile for this sample; Z-accumulator columns are constant
                v_sb[b] = vp.tile([128, CCH, N + 2], FP8, tag="v", name=f"v{b}")
                nc.gpsimd.dma_start(out=v_sb[b][:, :, N:N + 2], in_=vcol[:])
                et[b] = [None] * (CCH // 2)

            def v_chunk(b, m):
                # v_q = relu(Wv' @ x + t3) * (240/VMAX), fp8; scales folded.
                # Alternate the epilogue between ACT (1 op) and DVE (2 ops)
                # so neither engine becomes the per-iteration bottleneck.
                v_ps = psv.tile([128, 512], F32, tag="psv")
                for kp in range(CCH // 2):
                    nc.tensor.matmul(
                        v_ps[:, 0:N],
                        wvt[:, 2 * kp:2 * kp + 2, m * 128:(m + 1) * 128],
                        xq_sb[b][:, 2 * kp:2 * kp + 2, :],
                        start=(kp == 0), stop=(kp == CCH // 2 - 1),
                        perf_mode=DR,
                    )
                if m % 4 == 3:
                    nc.scalar.activation(
                        out=v_sb[b][:, m, 0:N], in_=v_ps[:, 0:N], func=AF.Relu,
                        bias=t3[:, m:m + 1], scale=vsc[:, m:m + 1],
                    )
                else:
                    vt = vtp.tile([128, N], BF16, tag="vt")
                    nc.vector.tensor_scalar(
                        out=vt, in0=v_ps[:, 0:N], scalar1=vsc[:, m:m + 1],
                        scalar2=t3[:, m:m + 1], op0=ALU.mult, op1=ALU.add,
                    )
                    nc.vector.tensor_scalar(
                        out=v_sb[b][:, m, 0:N], in0=vt, scalar1=0.0,
                        scalar2=None, op0=ALU.max,
                    )

            def sim_chunk(b, d):
                # et[d//2][:, d%2, c] = exp(ebias - sim[c, d]) in e5m2
                s_ps = psbig.tile([128, C], F32, tag="psbig")
                for cb in range(C // 512):
                    nc.tensor.matmul(
                        s_ps[:, cb * 512:(cb + 1) * 512],
                        qt_sb[b][:, d * 128:(d + 1) * 128],
                        k0[b][:, cb * 512:(cb + 1) * 512],
                        start=True, stop=True,
                    )
                if d % 2 == 0:
                    et[b][d // 2] = etp.tile([128, 2, C], FP8E5, tag="et",
                                             name=f"et{b}_{d // 2}")
                nc.scalar.activation(
                    out=et[b][d // 2][:, d % 2, :], in_=s_ps, func=AF.Exp,
                    bias=ebias[:, 0:1], scale=-1.0,
                )

            # u epilogues run one iteration behind their matmuls so the DVE
            # never reaches an STT whose psum is still accumulating (in-order
            # queue head-of-line blocking would stall the v-affine chain)
            u_pending = []
            u_osb = [None]

            def u_mm(b, m):
                # U = E @ v_ext
                u_ps = psu.tile([128, N + 2], F32, tag="psu",
                                name=f"ups{b}_{m}")
                for kp in range(CCH // 2):
                    nc.tensor.matmul(
                        u_ps,
                        et[b][kp][:, :, m * 128:(m + 1) * 128],
                        v_sb[b][:, 2 * kp:2 * kp + 2, :],
                        start=(kp == 0), stop=(kp == CCH // 2 - 1),
                        perf_mode=DR,
                    )
                u_pending.append((b, m, u_ps))

            def u_epi(keep):
                # out = U * (1/Z) + x fused on DVE, then stream the result out
                while len(u_pending) > keep:
                    b, m, u_ps = u_pending.pop(0)
                    if m % 2 == 0:
                        u_osb[0] = outp.tile([128, 2, N], F32, tag="o",
                                             name=f"o{b}_{m // 2}")
                    o_sb = u_osb[0]
                    rz = smallp.tile([128, 1], F32, tag="rz")
                    nc.vector.reciprocal(out=rz, in_=u_ps[:, N:N + 1])
                    nc.vector.scalar_tensor_tensor(
                        out=o_sb[:, m % 2, :],
                        in0=u_ps[:, 0:N],
                        scalar=rz[:, 0:1],
                        in1=x_sb[b][:, m, :],
                        op0=ALU.mult,
                        op1=ALU.add,
                    )
                    if m % 2 == 1:
                        nc.sync.dma_start(
                            out=out_d[b, :, m - 1:m + 1, :],
                            in_=o_sb,
                        )

            def sample(b, with_u):
                # qk(b) was already emitted by the previous sample's tail
                # (so its relu isn't queued behind the trailing exps)
                last = b == bs - 1
                for j in range(CCH):
                    # sim first so its exp leads the ACT queue each iteration
                    if j >= 2:
                        sim_chunk(b, j - 2)
                    if last and j == CCH - 1:
                        # finish the trailing sims early so the final
                        # u-phase does not wait on exp at the very end
                        sim_chunk(b, CCH - 2)
                        sim_chunk(b, CCH - 1)
                    # v chunks 0/1 were pre-emitted at the previous sample's
                    # boundary (PE filler while the trailing exps drain)
                    if j < CCH - 2:
                        v_chunk(b, j + 2)
                    if j == 0 and b + 1 < bs:
                        load_xt(b + 1)
                    if j == 1 and b + 1 < bs:
                        load_xq(b + 1)
                    if j == 3 and b + 1 < bs:
                        load_x(b + 1)
                    if with_u:
                        u_mm(b - 1, j)
                        u_epi(keep=0)
                if not last:
                    qk_phase(b + 1)
                    v_chunk(b + 1, 0)
                    v_chunk(b + 1, 1)
                    sim_chunk(b, CCH - 2)
                    sim_chunk(b, CCH - 1)

            load_xt(0)
            load_xq(0)
            load_x(0)
            qk_phase(0)
            v_chunk(0, 0)
            v_chunk(0, 1)
            for b in range(bs):
                sample(b, with_u=(b > 0))
            for j in range(CCH):
                u_mm(bs - 1, j)
                u_epi(keep=0)

    nc.compile()
    return nc


def _prep_host(x, Wq, Wk, Wv, bn1_g, bn1_b, bn1_m, bn1_v,
               bn2_g, bn2_b, bn2_m, bn2_v, bn3_g, bn3_b, bn3_m, bn3_v,
               abs_alpha):
    f = np.float32
    s1 = (bn1_g / np.sqrt(bn1_v + EPS)).astype(f)
    t1 = (bn1_b - s1 * bn1_m).astype(f)
    s2 = (bn2_g / np.sqrt(bn2_v + EPS)).astype(f)
    t2 = (bn2_b - s2 * bn2_m).astype(f)
    s3u = (bn3_g / np.sqrt(bn3_v + EPS)).astype(f)
    s3 = s3u * np.float32(abs_alpha)
    t3 = ((bn3_b - s3u * bn3_m) * abs_alpha).astype(f)

    x = np.asarray(x, dtype=f)
    sx = np.float32(240.0) / np.float32(np.abs(x).max() + 1e-30)

    # qk weights: per-row fp8 quant, dequant scale folded into the relu
    wqk = np.concatenate([Wq * s1[:, None], Wk * s2[:, None]], axis=0).astype(f)
    uqk = np.maximum(np.abs(wqk).max(axis=1) / 240.0, 1e-30).astype(f)
    wqk_q = np.clip(wqk / uqk[:, None], -240.0, 240.0).astype(F8NP)
    wqkt = np.ascontiguousarray(
        wqk_q.T.reshape(KCH, 128, 128).transpose(1, 0, 2))
    qsc = (uqk / sx).reshape(128, 1).astype(f)
    tqk = np.concatenate([t1, t2]).reshape(128, 1).astype(f)

    # v weights: per-row fp8 quant; global output scale 240/VMAX folded in
    vmax = np.float32(max(1.0, 40.0 * abs_alpha))
    vqs = np.float32(240.0) / vmax
    wv2 = (Wv * s3[:, None]).astype(f)
    uv = np.maximum(np.abs(wv2).max(axis=1) / 240.0, 1e-30).astype(f)
    wv_q = np.clip(wv2 / uv[:, None], -240.0, 240.0).astype(F8NP)
    wvt = np.ascontiguousarray(
        wv_q.T.reshape(CCH, 128, C).transpose(1, 0, 2))
    vsc = np.ascontiguousarray(
        ((uv / sx) * vqs).reshape(CCH, 128).T).astype(f)
    t3r = np.ascontiguousarray((t3 * vqs).reshape(CCH, 128).T).astype(f)
    ebias = np.full((128, 1),
                    -GAMMA + math.log(GCONST * vmax / 240.0), dtype=f)
    cst = np.concatenate([qsc, tqk, vsc, t3r, ebias], axis=1).astype(f)

    x8 = np.clip(x * sx, -240.0, 240.0).astype(F8NP)
    # residual [b, p(=c local), kc, n]
    xr = np.ascontiguousarray(x.reshape(B, CCH, 128, N).transpose(0, 2, 1, 3))
    xq8 = np.ascontiguousarray(x8.reshape(B, CCH, 128, N).transpose(0, 2, 1, 3))
    # [b, p(=n local), kc, c]
    xt8 = np.ascontiguousarray(
        x8.transpose(0, 2, 1).reshape(B, KCH, 128, C).transpose(0, 2, 1, 3))
    return xr, xq8, xt8, wqkt, wvt, cst, vqs


def kernel(x, Wq, Wk, Wv,
           bn1_g, bn1_b, bn1_m, bn1_v,
           bn2_g, bn2_b, bn2_m, bn2_v,
           bn3_g, bn3_b, bn3_m, bn3_v,
           alpha):
    global LAST_RESULTS
    args = [np.asarray(a, dtype=np.float32) for a in (
        x, Wq, Wk, Wv, bn1_g, bn1_b, bn1_m, bn1_v,
        bn2_g, bn2_b, bn2_m, bn2_v, bn3_g, bn3_b, bn3_m, bn3_v)]
    alpha_val = float(np.asarray(alpha).reshape(-1)[0])
    if alpha_val == 0.0:
        return np.asarray(x, dtype=np.float32).copy()

    xr, xq8, xt8, wqkt, wvt, cst, vqs = _prep_host(*args, abs(alpha_val))
    # Z-accumulator columns carry sign(alpha) * (240/VMAX) = sgn/s_v
    vcol = np.full((128, CCH, 2), np.sign(alpha_val) * vqs, dtype=F8NP)

    if "nc" not in _NC_CACHE:
        _NC_CACHE["nc"] = _build()
    nc = _NC_CACHE["nc"]

    in_maps = []
    for cid in range(NCORES):
        sl = slice(cid * BS, (cid + 1) * BS)
        in_maps.append({
            "x_in": np.ascontiguousarray(xr[sl]),
            "xq_in": np.ascontiguousarray(xq8[sl]),
            "xt_in": np.ascontiguousarray(xt8[sl]),
            "wqkt": wqkt,
            "wvt": wvt,
            "cst": cst,
            "vcol": vcol,
        })

    trace = bool(int(os.environ.get("KERNEL_TRACE", "0")))
    tmpdir = os.environ.get("KERNEL_TRACE_DIR") or None
    for attempt in range(3):
        res = run_bass_kernel_spmd(
            nc, in_maps, core_ids=list(range(NCORES)),
            trace=trace and attempt == 0, tmpdir=tmpdir,
        )
        LAST_RESULTS = res
        y = np.concatenate(
            [res.results[cid]["y_out"] for cid in range(NCORES)], axis=0)
        y = y.transpose(0, 2, 1, 3).reshape(B, C, N)
        if np.isfinite(y).all():
            break
        # rare transient NaN flake on device: rerun
    return np.ascontiguousarray(y.astype(np.float32))
